# revision 3
# baseline (speedup 1.0000x reference)
"""Distributed TRN2 kernel for nn_Att_scores — v9: v8 with AllGather (3KB/rank, half the ring time of AllReduce); the 8-way reduction folds into the u-matmul free dim + two tiny DVE reduces.

Same decomposition as v3/v4 (channel-sharded, one 6 KB AllReduce per
iteration, host sums the 8 partial-score outputs; bf16 inputs).

v5 delta: the engines execute their instruction streams IN ORDER, so in v4
every rep's post-AllReduce matmuls stalled on that rep's AllReduce with only
the warm-up block to hide behind (~5 us exposed latency per rep).  v5 emits
the unrolled reps as a software pipeline with lag D:

    block k emits:  pre-stage(k)  [loads, s-reduce, t_part, bounce, AR]
                    post-stage(k-D) [t load, t^T, u, partial scores, store]

so each AllReduce has ~D full iterations of other-rep work between issue and
first use.  Per-rep SBUF tiles rotate with bufs >= D+2.
"""

import numpy as np

B = 2
N = 2048
C = 768
H = 12
HD = C // H
SCALE = float(HD) ** -0.5
NCORES = 8
JB = C // NCORES          # 96 channels per core
JT = C // 128             # 6 chunks of 128 rows of Wq
NSEG = (B * N) // 512     # 8 output segments of 512 columns

_compiled_nc = None


def _build_and_compile(use_collective=True, repeats=1, warm=0, lag=3, ar_unused=False, ar_indep=False):
    import concourse.bass as bass  # noqa: F401
    import concourse.bacc as bacc
    import concourse.tile as tile
    import concourse.mybir as mybir
    from concourse import masks

    f32 = mybir.dt.float32
    bf16 = mybir.dt.bfloat16
    add = mybir.AluOpType.add
    copy_fn = mybir.ActivationFunctionType.Copy
    AX = mybir.AxisListType.X

    nc = bacc.Bacc(
        "TRN2",
        target_bir_lowering=False,
        debug=False,
        num_devices=NCORES,
    )

    x_d = nc.dram_tensor("x_in", [JB, B * N], bf16, kind="ExternalInput")
    wkt_d = nc.dram_tensor("wkt_in", [JB, C], bf16, kind="ExternalInput")
    wq_d = nc.dram_tensor("wq_in", [128, JT * JB], bf16, kind="ExternalInput")
    out_d = nc.dram_tensor("scores_out", [B, N], f32, kind="ExternalOutput")

    SLICES = ((0, 512), (512, 256))
    lag = min(lag, max(repeats - 1, 0))
    sbufs = lag + 2

    with tile.TileContext(nc) as tc:
        with (
            tc.tile_pool(name="sbuf", bufs=1) as pool,
            tc.tile_pool(name="psum", bufs=1, space="PSUM") as psum,
            tc.tile_pool(name="dram", bufs=1, space="DRAM") as dram,
        ):
            ones_red = pool.tile([128, 1], bf16)
            ident_f = pool.tile([B, B], f32)
            ident_b2 = pool.tile([B, B], bf16)
            ident_g = pool.tile([2 * NCORES, 2 * NCORES], bf16)
            nc.gpsimd.memset(ones_red[:], 1.0)
            masks.make_identity(nc, ident_f[:])
            masks.make_identity(nc, ident_b2[:])
            masks.make_identity(nc, ident_g[:])
            ar_static = dram.tile([1, B, C], bf16, name="ar_static")
            if ar_indep:
                zz = pool.tile([1, B * C], bf16, name="zz")
                nc.gpsimd.memset(zz[:], 0.0)
                nc.scalar.dma_start(
                    ar_static[:].rearrange("x b c -> x (b c)"), zz[:]
                )

            state = {}

            def pre_stage(rep):
                xc_sb = pool.tile([JB, B * N], bf16, tag="xc", bufs=sbufs)
                scratch = pool.tile([JB, 2048], bf16, tag="scr", bufs=2)
                wkt_sb = pool.tile([JB, C], bf16, tag="wkt", bufs=sbufs)
                wqc_sb = pool.tile([128, JT, JB], bf16, tag="wqc", bufs=sbufs)
                s_f = pool.tile([JB, 4], f32, tag="sf", bufs=2)
                s_bf = pool.tile([JB, B], bf16, tag="sbf", bufs=2)
                tp_sb = pool.tile([B, C], bf16, tag="tp", bufs=2)
                t_full = pool.tile([B * NCORES, C], bf16, tag="tf", bufs=sbufs)

                for q in range(4):
                    nc.sync.dma_start(
                        xc_sb[:, q * 1024 : (q + 1) * 1024],
                        x_d[:, q * 1024 : (q + 1) * 1024],
                    )
                nc.sync.dma_start(wkt_sb[:], wkt_d.ap())
                nc.sync.dma_start(
                    wqc_sb[:], wq_d.ap().rearrange("p (t f) -> p t f", t=JT)
                )

                nc.vector.reduce_sum(s_f[:, 0:1], xc_sb[:, 0:2048], axis=AX)
                nc.scalar.activation(
                    scratch[:, 0:2048], xc_sb[:, 2048:4096],
                    copy_fn, accum_out=s_f[:, 2:3],
                )
                nc.vector.tensor_copy(s_bf[:, 0:1], s_f[:, 0:1])
                nc.vector.tensor_copy(s_bf[:, 1:2], s_f[:, 2:3])

                tp_ps = psum.tile([B, 1024], f32, tag="tp", bufs=1, name="tp")
                for lo, nsz in SLICES:
                    nc.tensor.matmul(
                        tp_ps[:, lo : lo + nsz],
                        s_bf[:],
                        wkt_sb[:, lo : lo + nsz],
                        start=True,
                        stop=True,
                    )
                nc.scalar.copy(tp_sb[:, 0:512], tp_ps[:, 0:512])
                nc.vector.tensor_copy(tp_sb[:, 512:C], tp_ps[:, 512:C])

                ar_in = dram.tile([1, B, C], bf16, name=f"ar_in{rep}")
                ar_out = dram.tile(
                    [NCORES, B, C], bf16, addr_space="Shared", name=f"ar_out{rep}"
                )
                nc.scalar.dma_start(
                    ar_in[:].rearrange("x b c -> (x b) c"), tp_sb[:]
                )
                if use_collective:
                    nc.gpsimd.collective_compute(
                        "AllGather",
                        mybir.AluOpType.bypass,
                        replica_groups=[list(range(NCORES))],
                        ins=[(ar_static if ar_indep else ar_in).opt()],
                        outs=[ar_out.opt()],
                    )
                else:
                    for g in range(NCORES):
                        nc.scalar.dma_start(ar_out[g : g + 1], ar_in[:])
                if warm:
                    warm_ps = psum.tile([1, 512], f32, tag="wm", bufs=1)
                    for i in range(warm):
                        nc.tensor.matmul(
                            warm_ps[:],
                            ones_red[0:JB, :],
                            wkt_sb[:, 0:512],
                            start=(i == 0),
                            stop=(i == warm - 1),
                        )

                state[rep] = (xc_sb, wqc_sb, t_full, ar_in, ar_out)

            def post_stage(rep):
                xc_sb, wqc_sb, t_full, ar_in, ar_out = state.pop(rep)
                if ar_unused:
                    for g in range(NCORES):
                        nc.scalar.dma_start(
                            t_full[g * B : (g + 1) * B, :],
                            ar_in[:].rearrange("x b c -> (x b) c"),
                        )
                else:
                    nc.scalar.dma_start(
                        t_full[:], ar_out[:].rearrange("g b c -> (g b) c")
                    )
                tT_sb = pool.tile([128, JT, B * NCORES], bf16, tag="tT", bufs=2)
                uT_sb = pool.tile([JB, B], bf16, tag="uT", bufs=2)
                out_sb = pool.tile([B, B * N], bf16, tag="os", bufs=2)

                for ck in range(JT):
                    tt_ps = psum.tile(
                        [128, B * NCORES], bf16, tag="tr", bufs=2, name=f"tt{ck}"
                    )
                    nc.tensor.transpose(
                        tt_ps[:],
                        t_full[:, ck * 128 : (ck + 1) * 128],
                        ident_g[:],
                    )
                    nc.vector.tensor_copy(tT_sb[:, ck, :], tt_ps[:])

                u_ps = psum.tile(
                    [JB, B * NCORES], f32, tag="u", bufs=1, name="u"
                )
                for ck in range(JT):
                    nc.tensor.matmul(
                        u_ps[:],
                        wqc_sb[:, ck, :],
                        tT_sb[:, ck, :],
                        start=(ck == 0),
                        stop=(ck == JT - 1),
                    )
                # sum the 8 gathered contributions per batch (strided views)
                u_red = pool.tile([JB, B], f32, tag="ur", bufs=2)
                for b in range(B):
                    nc.vector.reduce_sum(
                        u_red[:, b : b + 1],
                        u_ps[:].rearrange("j (g b) -> j g b", b=B)[:, :, b],
                        axis=AX,
                    )
                nc.scalar.mul(uT_sb[:], u_red[:], SCALE)

                for k in range(NSEG):
                    ps_ps = psum.tile(
                        [B, 512], f32, tag="ps", bufs=2, name=f"ps{k}"
                    )
                    nc.tensor.matmul(
                        ps_ps[:],
                        uT_sb[:],
                        xc_sb[:, k * 512 : (k + 1) * 512],
                        start=True,
                        stop=True,
                    )
                    if k % 3 == 0:
                        nc.scalar.copy(
                            out_sb[:, k * 512 : (k + 1) * 512], ps_ps[:]
                        )
                    else:
                        nc.vector.tensor_copy(
                            out_sb[:, k * 512 : (k + 1) * 512], ps_ps[:]
                        )
                for b in range(B):
                    nc.gpsimd.dma_start(
                        out_d[b : b + 1, :],
                        out_sb[b : b + 1, b * N : (b + 1) * N],
                    )

            for rep in range(repeats + lag):
                if rep < repeats:
                    pre_stage(rep)
                if rep >= lag:
                    post_stage(rep - lag)

    nc.compile()
    return nc


def _get_nc():
    global _compiled_nc
    if _compiled_nc is None:
        _compiled_nc = _build_and_compile()
    return _compiled_nc


def make_in_maps(X, W_qkv):
    import ml_dtypes

    bf = ml_dtypes.bfloat16
    X = np.ascontiguousarray(X, dtype=np.float32)
    W = np.ascontiguousarray(W_qkv, dtype=np.float32)
    assert X.shape == (B, N, C) and W.shape == (2 * C, C)
    XT = np.ascontiguousarray(X.transpose(2, 0, 1).reshape(C, B * N)).astype(bf)
    maps = []
    for i in range(NCORES):
        ci = slice(i * JB, (i + 1) * JB)
        wkt = np.ascontiguousarray(W[C : 2 * C, ci].T).astype(bf)    # [96, 768]
        wq = W[0:C, ci]                                              # [768, 96]
        wq_sw = np.ascontiguousarray(
            wq.reshape(JT, 128, JB).transpose(1, 0, 2).reshape(128, JT * JB)
        ).astype(bf)
        maps.append(
            {
                "x_in": np.ascontiguousarray(XT[ci]),
                "wkt_in": wkt,
                "wq_in": wq_sw,
            }
        )
    return maps


def assemble_out(results):
    acc = results[0]["scores_out"].astype(np.float32).copy()
    for i in range(1, NCORES):
        acc += results[i]["scores_out"]
    return acc


def kernel(X, W_qkv):
    from concourse import bass_utils

    nc = _get_nc()
    res = bass_utils.run_bass_kernel_spmd(
        nc, make_in_maps(X, W_qkv), core_ids=list(range(NCORES))
    )
    return assemble_out(res.results)


# revision 4
# speedup vs baseline: 2.1062x; 2.1062x over previous
"""Distributed TRN2 kernel for nn_Att_scores — v9b (ps bufs=3): v8 with AllGather (3KB/rank, half the ring time of AllReduce); the 8-way reduction folds into the u-matmul free dim + two tiny DVE reduces.

Same decomposition as v3/v4 (channel-sharded, one 6 KB AllReduce per
iteration, host sums the 8 partial-score outputs; bf16 inputs).

v5 delta: the engines execute their instruction streams IN ORDER, so in v4
every rep's post-AllReduce matmuls stalled on that rep's AllReduce with only
the warm-up block to hide behind (~5 us exposed latency per rep).  v5 emits
the unrolled reps as a software pipeline with lag D:

    block k emits:  pre-stage(k)  [loads, s-reduce, t_part, bounce, AR]
                    post-stage(k-D) [t load, t^T, u, partial scores, store]

so each AllReduce has ~D full iterations of other-rep work between issue and
first use.  Per-rep SBUF tiles rotate with bufs >= D+2.
"""

import numpy as np

B = 2
N = 2048
C = 768
H = 12
HD = C // H
SCALE = float(HD) ** -0.5
NCORES = 8
JB = C // NCORES          # 96 channels per core
JT = C // 128             # 6 chunks of 128 rows of Wq
NSEG = (B * N) // 512     # 8 output segments of 512 columns

_compiled_nc = None


def _build_and_compile(use_collective=True, repeats=1, warm=0, lag=5, ar_unused=False, ar_indep=False, nslab=2):
    import concourse.bass as bass  # noqa: F401
    import concourse.bacc as bacc
    import concourse.tile as tile
    import concourse.mybir as mybir
    from concourse import masks

    f32 = mybir.dt.float32
    bf16 = mybir.dt.bfloat16
    add = mybir.AluOpType.add
    copy_fn = mybir.ActivationFunctionType.Copy
    AX = mybir.AxisListType.X

    nc = bacc.Bacc(
        "TRN2",
        target_bir_lowering=False,
        debug=False,
        num_devices=NCORES,
    )

    x_d = nc.dram_tensor("x_in", [JB, B * N], bf16, kind="ExternalInput")
    wkt_d = nc.dram_tensor("wkt_in", [JB, C], bf16, kind="ExternalInput")
    wq_d = nc.dram_tensor("wq_in", [128, JT * JB], bf16, kind="ExternalInput")
    out_d = nc.dram_tensor("scores_out", [B, N], f32, kind="ExternalOutput")

    SLICES = ((0, 512), (512, 256))
    lag = min(lag, max(repeats - 1, 0))
    sbufs = lag + 2

    with tile.TileContext(nc) as tc:
        with (
            tc.tile_pool(name="sbuf", bufs=1) as pool,
            tc.tile_pool(name="psum", bufs=1, space="PSUM") as psum,
            tc.tile_pool(name="dram", bufs=1, space="DRAM") as dram,
        ):
            ones_red = pool.tile([128, 1], bf16)
            ident_f = pool.tile([B, B], f32)
            ident_b2 = pool.tile([B, B], bf16)
            ident_g = pool.tile([2 * NCORES, 2 * NCORES], bf16)
            nc.gpsimd.memset(ones_red[:], 1.0)
            masks.make_identity(nc, ident_f[:])
            masks.make_identity(nc, ident_b2[:])
            masks.make_identity(nc, ident_g[:])
            ar_static = dram.tile([1, B, C], bf16, name="ar_static")
            if ar_indep:
                zz = pool.tile([1, B * C], bf16, name="zz")
                nc.gpsimd.memset(zz[:], 0.0)
                nc.scalar.dma_start(
                    ar_static[:].rearrange("x b c -> x (b c)"), zz[:]
                )

            state = {}

            def pre_stage(rep):
                xc_sb = pool.tile([JB, B * N], bf16, tag="xc", bufs=sbufs)
                scratch = pool.tile([JB, 2048], bf16, tag="scr", bufs=2)
                wkt_sb = pool.tile([JB, C], bf16, tag="wkt", bufs=sbufs)
                wqc_sb = pool.tile([128, JT, JB], bf16, tag="wqc", bufs=sbufs)
                s_f = pool.tile([JB, 4], f32, tag="sf", bufs=2)
                s_bf = pool.tile([JB, B], bf16, tag="sbf", bufs=2)
                tp_sb = pool.tile([B, C], bf16, tag="tp", bufs=2)
                t_full = pool.tile([B * NCORES, C], bf16, tag="tf", bufs=sbufs)

                slab = (B * N) // nslab
                for q in range(nslab):
                    nc.sync.dma_start(
                        xc_sb[:, q * slab : (q + 1) * slab],
                        x_d[:, q * slab : (q + 1) * slab],
                    )
                nc.sync.dma_start(wkt_sb[:], wkt_d.ap())
                nc.sync.dma_start(
                    wqc_sb[:], wq_d.ap().rearrange("p (t f) -> p t f", t=JT)
                )

                nc.vector.reduce_sum(s_f[:, 0:1], xc_sb[:, 0:2048], axis=AX)
                nc.scalar.activation(
                    scratch[:, 0:2048], xc_sb[:, 2048:4096],
                    copy_fn, accum_out=s_f[:, 2:3],
                )
                nc.vector.tensor_copy(s_bf[:, 0:1], s_f[:, 0:1])
                nc.vector.tensor_copy(s_bf[:, 1:2], s_f[:, 2:3])

                tp_ps = psum.tile([B, 1024], f32, tag="tp", bufs=1, name="tp")
                for lo, nsz in SLICES:
                    nc.tensor.matmul(
                        tp_ps[:, lo : lo + nsz],
                        s_bf[:],
                        wkt_sb[:, lo : lo + nsz],
                        start=True,
                        stop=True,
                    )
                nc.scalar.copy(tp_sb[:, 0:512], tp_ps[:, 0:512])
                nc.vector.tensor_copy(tp_sb[:, 512:C], tp_ps[:, 512:C])

                ar_in = dram.tile([1, B, C], bf16, name=f"ar_in{rep}")
                ar_out = dram.tile(
                    [NCORES, B, C], bf16, addr_space="Shared", name=f"ar_out{rep}"
                )
                nc.scalar.dma_start(
                    ar_in[:].rearrange("x b c -> (x b) c"), tp_sb[:]
                )
                if use_collective:
                    nc.gpsimd.collective_compute(
                        "AllGather",
                        mybir.AluOpType.bypass,
                        replica_groups=[list(range(NCORES))],
                        ins=[(ar_static if ar_indep else ar_in).opt()],
                        outs=[ar_out.opt()],
                    )
                else:
                    for g in range(NCORES):
                        nc.scalar.dma_start(ar_out[g : g + 1], ar_in[:])
                if warm:
                    warm_ps = psum.tile([1, 512], f32, tag="wm", bufs=1)
                    for i in range(warm):
                        nc.tensor.matmul(
                            warm_ps[:],
                            ones_red[0:JB, :],
                            wkt_sb[:, 0:512],
                            start=(i == 0),
                            stop=(i == warm - 1),
                        )

                state[rep] = (xc_sb, wqc_sb, t_full, ar_in, ar_out)

            def post_stage(rep):
                xc_sb, wqc_sb, t_full, ar_in, ar_out = state.pop(rep)
                if ar_unused:
                    for g in range(NCORES):
                        nc.scalar.dma_start(
                            t_full[g * B : (g + 1) * B, :],
                            ar_in[:].rearrange("x b c -> (x b) c"),
                        )
                else:
                    nc.scalar.dma_start(
                        t_full[:], ar_out[:].rearrange("g b c -> (g b) c")
                    )
                tT_sb = pool.tile([128, JT, B * NCORES], bf16, tag="tT", bufs=2)
                uT_sb = pool.tile([JB, B], bf16, tag="uT", bufs=2)
                out_sb = pool.tile([B, B * N], bf16, tag="os", bufs=2)

                for ck in range(JT):
                    tt_ps = psum.tile(
                        [128, B * NCORES], bf16, tag="tr", bufs=2, name=f"tt{ck}"
                    )
                    nc.tensor.transpose(
                        tt_ps[:],
                        t_full[:, ck * 128 : (ck + 1) * 128],
                        ident_g[:],
                    )
                    nc.vector.tensor_copy(tT_sb[:, ck, :], tt_ps[:])

                u_ps = psum.tile(
                    [JB, B * NCORES], f32, tag="u", bufs=1, name="u"
                )
                for ck in range(JT):
                    nc.tensor.matmul(
                        u_ps[:],
                        wqc_sb[:, ck, :],
                        tT_sb[:, ck, :],
                        start=(ck == 0),
                        stop=(ck == JT - 1),
                    )
                # sum the 8 gathered contributions per batch (strided views)
                u_red = pool.tile([JB, B], f32, tag="ur", bufs=2)
                for b in range(B):
                    nc.vector.reduce_sum(
                        u_red[:, b : b + 1],
                        u_ps[:].rearrange("j (g b) -> j g b", b=B)[:, :, b],
                        axis=AX,
                    )
                nc.scalar.mul(uT_sb[:], u_red[:], SCALE)

                for k in range(NSEG):
                    ps_ps = psum.tile(
                        [B, 512], f32, tag="ps", bufs=3, name=f"ps{k}"
                    )
                    nc.tensor.matmul(
                        ps_ps[:],
                        uT_sb[:],
                        xc_sb[:, k * 512 : (k + 1) * 512],
                        start=True,
                        stop=True,
                    )
                    if k % 3 == 0:
                        nc.scalar.copy(
                            out_sb[:, k * 512 : (k + 1) * 512], ps_ps[:]
                        )
                    else:
                        nc.vector.tensor_copy(
                            out_sb[:, k * 512 : (k + 1) * 512], ps_ps[:]
                        )
                for b in range(B):
                    nc.gpsimd.dma_start(
                        out_d[b : b + 1, :],
                        out_sb[b : b + 1, b * N : (b + 1) * N],
                    )

            for rep in range(repeats + lag):
                if rep < repeats:
                    pre_stage(rep)
                if rep >= lag:
                    post_stage(rep - lag)

    nc.compile()
    return nc


def _get_nc():
    global _compiled_nc
    if _compiled_nc is None:
        _compiled_nc = _build_and_compile()
    return _compiled_nc


def make_in_maps(X, W_qkv):
    import ml_dtypes

    bf = ml_dtypes.bfloat16
    X = np.ascontiguousarray(X, dtype=np.float32)
    W = np.ascontiguousarray(W_qkv, dtype=np.float32)
    assert X.shape == (B, N, C) and W.shape == (2 * C, C)
    XT = np.ascontiguousarray(X.transpose(2, 0, 1).reshape(C, B * N)).astype(bf)
    maps = []
    for i in range(NCORES):
        ci = slice(i * JB, (i + 1) * JB)
        wkt = np.ascontiguousarray(W[C : 2 * C, ci].T).astype(bf)    # [96, 768]
        wq = W[0:C, ci]                                              # [768, 96]
        wq_sw = np.ascontiguousarray(
            wq.reshape(JT, 128, JB).transpose(1, 0, 2).reshape(128, JT * JB)
        ).astype(bf)
        maps.append(
            {
                "x_in": np.ascontiguousarray(XT[ci]),
                "wkt_in": wkt,
                "wq_in": wq_sw,
            }
        )
    return maps


def assemble_out(results):
    acc = results[0]["scores_out"].astype(np.float32).copy()
    for i in range(1, NCORES):
        acc += results[i]["scores_out"]
    return acc


def kernel(X, W_qkv):
    from concourse import bass_utils

    nc = _get_nc()
    res = bass_utils.run_bass_kernel_spmd(
        nc, make_in_maps(X, W_qkv), core_ids=list(range(NCORES))
    )
    return assemble_out(res.results)
